# revision 3
# baseline (speedup 1.0000x reference)
"""Multi-class 3D DICE loss on 8 Trainium2 NeuronCores — reduced-precision
streaming.

The loss only needs three per-(subject, class) reductions over 1M-element
volumes: inter = sum(x*m), msum = sum(m), xsum = sum(x); the ~10-flop DICE
tail runs on the host. Random-rounding error on sums of 1M uniform values
averages out (~1e-6 relative on the loss, HW-verified), so the HBM stream —
the entire cost of this memory-bound kernel — runs below fp32:

  - x (probs) staged as bf16 (RNE, exact-format match with device decode)
  - m (masks) staged as bf16 or fp8e3 (E3M4; masks only feed sums and the
    product, never a divide) — M_FMT below.

HW-microbenchmarked facts this design is built on (see microbench.py):
  - Any DVE op with accum_out runs at 1x (the side accumulator path);
    ScalarE is always 1x; PE matmul is ~477-640ns per 512-col slice
    regardless of dtype. So each engine can carry ~one full reduction
    stream and the kernel must spread the three quantities across DVE
    (inter), ScalarE (msum), PE (xsum).
  - TRN2 has TWO hardware DGE queues (SP/sync + Activation/scalar).
    One queue saturates ~280 GB/s with engines busy; both together were
    measured ~480 GB/s. Chunks are greedily split across the queues by
    cumulative bytes. 16 KB/partition descriptors measured fastest.
  - Dummy stride-0 outputs for the accumulating ops save SBUF write
    bandwidth, which the DMA stream needs.

Sharding: 1024 units (one unit = 256 columns of the per-subject
[128, 32768] view) split by parity — even cores 127 units, odd cores 129 —
as two compiled variants on disjoint jax meshes; a sporadic late-stream DMA
stall only ever hits even cores, so evens get the smaller share. Each
core's shard is 1-3 contiguous segments, each inside one subject; segment
boundaries are placed so the big PSUM folds run mid-stream, off the tail.
The host regroups segments by subject and applies the DICE tail.
"""

import math
import os
import sys
from contextlib import ExitStack

import numpy as np

for _p in ("/opt/trn_rl_repo",):
    if _p not in sys.path and os.path.isdir(_p):
        sys.path.insert(0, _p)

import ml_dtypes  # noqa: E402

import concourse.tile as tile  # noqa: E402
from concourse import bacc, bass2jax, mybir  # noqa: E402

N_CORES = 8
B, C = 8, 4
SPATIAL = 64 * 128 * 128            # 1,048,576 per (subject, class)
P = 128                             # SBUF partitions = C * 32
SUBJ_COLS = (C * SPATIAL) // P      # 32768 columns per subject
UNIT = 256                          # shard granularity
SUBJ_UNITS = SUBJ_COLS // UNIT      # 128
MM = 512                            # matmul slice (max moving free dim)
EPS = 1e-7
F32 = mybir.dt.float32
BF16 = mybir.dt.bfloat16
U8 = mybir.dt.uint8

# Mask-tensor wire format: "bf16" (2 B/col) or "fp8e3" (1 B/col, E3M4).
M_FMT = "bf16"
X_BYTES = 2
M_BYTES = {"bf16": 2, "fp8e3": 1}[M_FMT]
BPC = X_BYTES + M_BYTES             # wire bytes per column (x + m)
M_DT = {"bf16": BF16, "fp8e3": mybir.dt.float8e3}[M_FMT]

# Variant name -> (chunk schedule in columns, chunks-per-segment). Chunk
# boundaries are aligned so no chunk straddles a segment boundary; 4096-col
# steady chunks give 16KB/partition descriptors (measured-best DMA rate)
# and a small run tail.
VARIANTS = {
    "vodd": dict(
        # spare unit FIRST so the run tail is the main segment's small
        # closing chunks; last segment tiny so its PSUM fold is cheap.
        chunks=[256, 4096, 4096, 4096, 4096, 4096, 4096, 4096, 2048, 1024,
                512, 512],
        seg_nchunks=(1, 10, 1),
    ),
    "veven": dict(
        chunks=[4096, 4096, 4096, 4096, 4096, 4096, 4096, 2048, 1024, 512,
                256],
        seg_nchunks=(10, 1),
    ),
}

# core -> (variant, [(subject, unit_start, n_units), ...]) in SEGMENT ORDER.
# The sporadic late-stream stall only ever hits EVEN cores; evens get 127
# units to odds' 129 so evens finish clear of it.
ASSIGN = {
    0: ("veven", [(0, 0, 126), (0, 126, 1)]),
    2: ("veven", [(2, 0, 126), (2, 126, 1)]),
    4: ("veven", [(4, 0, 126), (4, 126, 1)]),
    6: ("veven", [(6, 0, 126), (6, 126, 1)]),
    1: ("vodd", [(0, 127, 1), (1, 0, 126), (1, 126, 2)]),
    3: ("vodd", [(2, 127, 1), (3, 0, 126), (3, 126, 2)]),
    5: ("vodd", [(4, 127, 1), (5, 0, 126), (5, 126, 2)]),
    7: ("vodd", [(6, 127, 1), (7, 0, 126), (7, 126, 2)]),
}
GROUPS = {  # variant -> device ids; evens dispatched first (odds get the
    # later slot — dispatch-order overhead lands on the slack-rich group)
    "veven": [0, 2, 4, 6],
    "vodd": [1, 3, 5, 7],
}


def _queue_split(chunks):
    """Greedy split of chunk indices across the two HWDGE queues by bytes."""
    loads = [0, 0]
    which = []
    for fd in chunks:
        q = 0 if loads[0] <= loads[1] else 1
        which.append(q)
        loads[q] += fd
    return which


def _check_assign():
    cover = np.zeros((B, SUBJ_UNITS), dtype=int)
    for core, (vname, segs) in ASSIGN.items():
        v = VARIANTS[vname]
        starts = [sum(v["seg_nchunks"][:i]) for i in range(len(v["seg_nchunks"]))]
        seg_cols = [
            sum(v["chunks"][a : a + ns])
            for a, ns in zip(starts, v["seg_nchunks"])
        ]
        assert len(segs) == len(seg_cols)
        for (sub, us, n), cols in zip(segs, seg_cols):
            assert n * UNIT == cols, (core, vname, n * UNIT, cols)
            cover[sub, us : us + n] += 1
    assert (cover == 1).all()


_check_assign()


def _dice_body(ctx, tc, out_ap, x_ap, ind_ap, chunks, seg_nchunks):
    nc = tc.nc
    add = mybir.AluOpType.add
    mult = mybir.AluOpType.mult
    Copy = mybir.ActivationFunctionType.Copy
    NCH = len(chunks)
    NSEG = len(seg_nchunks)
    PADN = 8 * math.ceil(NCH / 8)  # keep each engine's accum cols in own 32B words
    seg_start = [sum(seg_nchunks[:i]) for i in range(NSEG)]  # first chunk of seg
    seg_of = []
    for s, ns in enumerate(seg_nchunks):
        seg_of += [s] * ns
    qs = _queue_split(chunks)

    def _n_slices(fd):
        return (fd + MM - 1) // MM

    seg_slices = [
        sum(_n_slices(c) for c in chunks[seg_start[s] : seg_start[s] + ns])
        for s, ns in enumerate(seg_nchunks)
    ]

    consts = ctx.enter_context(tc.tile_pool(name="consts", bufs=1))
    # Per-queue chunk pools: 3 buffers of run-ahead each.
    qpool = [
        ctx.enter_context(tc.tile_pool(name="xq0", bufs=3)),
        ctx.enter_context(tc.tile_pool(name="xq1", bufs=3)),
    ]
    mpool = ctx.enter_context(tc.tile_pool(name="m16", bufs=3))
    small = ctx.enter_context(tc.tile_pool(name="small", bufs=1))
    psum = ctx.enter_context(tc.tile_pool(name="psum", bufs=1, space="PSUM"))

    # Block indicator: ind[q, c] = 1.0 iff q // 32 == c. f32 copy (memset)
    # for the final fp32 collapse; bf16 copy (DMA) as lhsT for the
    # per-chunk x-sum matmuls. Both exact.
    ind = consts.tile([P, C], F32)
    nc.vector.memset(ind[:], 0.0)
    for c in range(C):
        nc.vector.memset(ind[c * 32 : (c + 1) * 32, c : c + 1], 1.0)
    # Loaded after chunk 0's input DMA is issued — it is only needed by the
    # first matmul, which waits on chunk 0's data anyway.
    ind_b = consts.tile([P, C], BF16, tag="ind_b")

    # Per-chunk partial sums (column j <- chunk j); no cross-chunk deps.
    # Cols [0,PADN) = inter on DVE, [PADN,2*PADN) = msum on ScalarE — each
    # engine owns full 32 B accumulator words (mixing engines within one
    # word produced lost-update corruption on HW). Zero the pad columns so
    # the collapse matmul never reads uninitialized SBUF.
    acc = small.tile([P, 2 * PADN], F32)
    nc.vector.memset(acc[:], 0.0)
    # Accumulating ops run at 1x regardless of output AP (HW-measured), so
    # use stride-0 dummies and save the SBUF write bandwidth for DMA.
    dve_dummy = small.tile([P, 1], F32)
    act_dummy = small.tile([P, 1], F32)
    sums = small.tile([C, 3 * NSEG], F32, tag="sums")
    # PE accumulates per-class x column sums across each segment's slices.
    ps_x = []
    for s in range(NSEG):
        seg_cols = sum(chunks[seg_start[s] : seg_start[s] + seg_nchunks[s]])
        ps_x_s = psum.tile([C, min(MM, seg_cols)], F32, tag=f"ps_x{s}")
        ps_x.append(ps_x_s)

    off = 0
    sl_in_seg = 0
    for j, fd in enumerate(chunks):
        seg = seg_of[j]
        if j > 0 and seg_of[j - 1] != seg:
            sl_in_seg = 0
        # One fused DMA per chunk delivers [x_bytes | m_bytes]; chunks are
        # split across the two hardware DGE queues (sync + scalar).
        xmt = qpool[qs[j]].tile([P, BPC * fd], U8, tag="xmt")
        if qs[j] == 0:
            nc.sync.dma_start(out=xmt[:], in_=x_ap[:, BPC * off : BPC * (off + fd)])
        else:
            nc.scalar.dma_start(out=xmt[:], in_=x_ap[:, BPC * off : BPC * (off + fd)])
        xt = xmt[:, : X_BYTES * fd].bitcast(BF16)           # [P, fd] bf16
        mt = xmt[:, X_BYTES * fd :].bitcast(M_DT)           # [P, fd] m-format
        off += fd
        if j == 0:
            nc.sync.dma_start(out=ind_b[:], in_=ind_ap[:])

        # ScalarE: msum partial via accum_out; in fp8 mode the same pass is
        # the upcast the DVE needs (out becomes a real bf16 scratch).
        if M_FMT == "bf16":
            nc.scalar.activation(
                out=act_dummy.broadcast_to((P, fd)),
                in_=mt,
                func=Copy,
                accum_out=acc[:, PADN + j : PADN + j + 1],
            )
            m_in = mt
        else:
            m16 = mpool.tile([P, max(chunks)], BF16, tag="m16")
            nc.scalar.activation(
                out=m16[:, :fd],
                in_=mt,
                func=Copy,
                accum_out=acc[:, PADN + j : PADN + j + 1],
            )
            m_in = m16[:, :fd]

        # DVE: inter partial: accum = X-reduce((x*1)*m).
        nc.vector.scalar_tensor_tensor(
            out=dve_dummy.broadcast_to((P, fd)),
            in0=xt,
            scalar=1.0,
            in1=m_in,
            op0=mult,
            op1=mult,
            accum_out=acc[:, j : j + 1],
        )
        # x-sums on PE: ps_x[seg][c, i] += sum_q ind[q, c] * x[q, s*MM+i],
        # accumulated in PSUM across the segment's slices.
        for s in range(_n_slices(fd)):
            w = min(MM, fd - s * MM)
            nc.tensor.matmul(
                out=ps_x[seg][:, :w],
                lhsT=ind_b[:],
                rhs=xt[:, s * MM : s * MM + w],
                start=(sl_in_seg == 0),
                stop=(sl_in_seg == seg_slices[seg] - 1),
            )
            sl_in_seg += 1
        # Segment finished: fold its PSUM x-sums now, while the stream
        # continues — keeps the [C,512] reduce off the run tail.
        if j == seg_start[seg] + seg_nchunks[seg] - 1:
            nc.vector.tensor_reduce(
                sums[:, 3 * seg + 2 : 3 * seg + 3],
                ps_x[seg][:],
                axis=mybir.AxisListType.X,
                op=add,
            )

    # Partition blocks -> per-(class, quantity, chunk) sums in one matmul,
    # then per-segment PSUM-side reduces -> [C, 3*NSEG] segment sums
    # (inter, msum, xsum per segment). The remaining ~10-flop scalar tail
    # runs on the host during unshard.
    ps2 = psum.tile([C, 2 * PADN], F32)
    nc.tensor.matmul(out=ps2[:], lhsT=ind[:], rhs=acc[:], start=True, stop=True)
    for s, ns in enumerate(seg_nchunks):
        a = seg_start[s]
        nc.vector.tensor_reduce(
            sums[:, 3 * s : 3 * s + 1],
            ps2[:, a : a + ns],
            axis=mybir.AxisListType.X,
            op=add,
        )
        nc.vector.tensor_reduce(
            sums[:, 3 * s + 1 : 3 * s + 2],
            ps2[:, PADN + a : PADN + a + ns],
            axis=mybir.AxisListType.X,
            op=add,
        )
    nc.sync.dma_start(out=out_ap, in_=sums[:])


_CACHE: dict[str, object] = {}


def _build(vname: str):
    key = f"nc_{vname}"
    if key in _CACHE:
        return _CACHE[key]
    v = VARIANTS[vname]
    cols = sum(v["chunks"])
    nseg = len(v["seg_nchunks"])
    nc = bacc.Bacc("TRN2", target_bir_lowering=False, debug=False)
    xm = nc.dram_tensor("xm", [P, BPC * cols], U8, kind="ExternalInput").ap()
    ind = nc.dram_tensor("ind", [P, C], BF16, kind="ExternalInput").ap()
    out = nc.dram_tensor("seg_sums", [C, 3 * nseg], F32, kind="ExternalOutput").ap()
    with tile.TileContext(nc) as tc:
        with ExitStack() as ctx:
            _dice_body(ctx, tc, out, xm, ind, v["chunks"], v["seg_nchunks"])
    nc.compile()
    _CACHE[key] = nc
    return nc


def _runner(vname: str):
    """Jitted shard_map runner for a variant on its assigned devices."""
    key = f"run_{vname}"
    if key in _CACHE:
        return _CACHE[key]
    import jax
    from jax.experimental.shard_map import shard_map
    from jax.sharding import Mesh, PartitionSpec

    bass2jax.install_neuronx_cc_hook()
    nc = _build(vname)
    device_ids = GROUPS[vname]

    partition_name = (
        nc.partition_id_tensor.name if nc.partition_id_tensor else None
    )
    in_names, out_names, out_avals, zero_outs = [], [], [], []
    for alloc in nc.m.functions[0].allocations:
        if not isinstance(alloc, mybir.MemoryLocationSet):
            continue
        name = alloc.memorylocations[0].name
        if alloc.kind == "ExternalInput":
            if name != partition_name:
                in_names.append(name)
        elif alloc.kind == "ExternalOutput":
            out_names.append(name)
            shape = tuple(alloc.tensor_shape)
            dtype = mybir.dt.np(alloc.dtype)
            out_avals.append(jax.core.ShapedArray(shape, dtype))
            zero_outs.append(np.zeros(shape, dtype))
    n_params = len(in_names)
    n_outs = len(out_avals)
    all_in_names = in_names + out_names
    if partition_name is not None:
        all_in_names.append(partition_name)
    donate = tuple(range(n_params, n_params + n_outs))

    def _body(*args):
        operands = list(args)
        if partition_name is not None:
            operands.append(bass2jax.partition_id_tensor())
        outs = bass2jax._bass_exec_p.bind(
            *operands,
            out_avals=tuple(out_avals),
            in_names=tuple(all_in_names),
            out_names=tuple(out_names),
            lowering_input_output_aliases=(),
            sim_require_finite=True,
            sim_require_nnan=True,
            nc=nc,
        )
        return tuple(outs)

    devices = [jax.devices()[i] for i in device_ids]
    n = len(devices)
    mesh = Mesh(np.asarray(devices), ("core",))
    in_specs = (PartitionSpec("core"),) * (n_params + n_outs)
    out_specs = (PartitionSpec("core"),) * n_outs
    sharded = jax.jit(
        shard_map(_body, mesh=mesh, in_specs=in_specs, out_specs=out_specs,
                  check_rep=False),
        donate_argnums=donate,
        keep_unused=True,
    )

    def run(in_maps):
        assert len(in_maps) == n
        per_core = [[np.asarray(m_[nm]) for nm in in_names] for m_ in in_maps]
        concat_in = [
            np.concatenate([per_core[c][i] for c in range(n)], axis=0)
            for i in range(n_params)
        ]
        concat_zeros = [
            np.zeros((n * z.shape[0], *z.shape[1:]), z.dtype) for z in zero_outs
        ]
        out_arrs = sharded(*concat_in, *concat_zeros)

        def gather():
            return [
                {
                    name: np.asarray(out_arrs[i]).reshape(n, *out_avals[i].shape)[c]
                    for i, name in enumerate(out_names)
                }
                for c in range(n)
            ]

        return gather

    _CACHE[key] = run
    return run


_IND_NP = np.repeat(np.eye(C, dtype=np.float32), 32, axis=0).astype(
    ml_dtypes.bfloat16
)  # [128, 4]


def _stage(output: np.ndarray, masks: np.ndarray):
    """Whole-tensor dtype staging, done once per kernel() call."""
    x16 = (
        np.ascontiguousarray(output, dtype=np.float32)
        .astype(ml_dtypes.bfloat16)
        .reshape(B, P, SUBJ_COLS)
    )
    m_np = {"bf16": ml_dtypes.bfloat16, "fp8e3": ml_dtypes.float8_e3m4}[M_FMT]
    m8 = (
        np.ascontiguousarray(masks, dtype=np.float32)
        .astype(m_np)
        .reshape(B, P, SUBJ_COLS)
    )
    return x16, m8


def _core_inputs(x16: np.ndarray, m8: np.ndarray, core: int):
    vname, segs = ASSIGN[core]
    xs, ms = [], []
    for sub, us, n in segs:
        lo, hi = us * UNIT, (us + n) * UNIT
        xs.append(x16[sub, :, lo:hi])
        ms.append(m8[sub, :, lo:hi])
    x = np.ascontiguousarray(np.concatenate(xs, axis=1)).view(np.uint8)
    m = np.ascontiguousarray(np.concatenate(ms, axis=1)).view(np.uint8)
    # Pack per DMA chunk: [x_chunk_bytes | m_chunk_bytes] so one transfer
    # feeds both operands.
    chunks = VARIANTS[vname]["chunks"]
    cols = x.shape[1] // X_BYTES
    xm = np.empty((P, BPC * cols), dtype=np.uint8)
    off = 0
    for fd in chunks:
        dst = BPC * off
        xm[:, dst : dst + X_BYTES * fd] = x[:, X_BYTES * off : X_BYTES * (off + fd)]
        xm[:, dst + X_BYTES * fd : dst + BPC * fd] = m[
            :, M_BYTES * off : M_BYTES * (off + fd)
        ]
        off += fd
    return {"xm": xm, "ind": _IND_NP}


def run_split(output: np.ndarray, masks: np.ndarray):
    """Dispatch both variants concurrently; returns (loss[1], groups)
    where groups = [(vname, nc, device_ids)] for the profiler."""
    x16, m8 = _stage(output, masks)

    def _dispatch_all():
        gathers = []
        for vname, ids in GROUPS.items():
            run = _runner(vname)
            gathers.append(
                (vname, ids, run([_core_inputs(x16, m8, c) for c in ids]))
            )
        # force completion inside the retry scope
        return [(v, ids, g()) for v, ids, g in gathers]

    try:
        finished = _dispatch_all()
    except Exception:  # e.g. a wedged NeuronCore from a prior run — retry once
        import time as _time

        _time.sleep(10)
        finished = _dispatch_all()

    # [B, C, 3] per-subject class sums assembled from segment partials.
    subj = np.zeros((B, C, 3), dtype=np.float32)
    for vname, ids, results in finished:
        for slot, core in enumerate(ids):
            _, segs = ASSIGN[core]
            seg_sums = results[slot]["seg_sums"].astype(np.float32)  # [C, 3*NSEG]
            for s, (sub, _, _) in enumerate(segs):
                subj[sub] += seg_sums[:, 3 * s : 3 * s + 3]

    per_subj = np.array([_finish(subj[b]) for b in range(B)], dtype=np.float32)
    loss = (per_subj.sum(dtype=np.float32) / np.float32(B)).reshape(1)
    groups = [(vname, _CACHE[f"nc_{vname}"], ids) for vname, ids in GROUPS.items()]
    return loss.astype(np.float32), groups


def _finish(cs: np.ndarray) -> np.float32:
    """Per-subject scalar tail (fp32, mirrors the reference ordering).

    cs: [C, 3] — columns (inter, mask_sum, x_sum) per class.
    """
    cs = cs.astype(np.float32)
    inter, msum, xsum = cs[:, 0], cs[:, 1], cs[:, 2]
    w = np.float32(1.0) / (msum * msum + np.float32(EPS))
    total = xsum + msum
    nom = (w * inter).sum(dtype=np.float32)
    den = (w * total + np.float32(EPS)).sum(dtype=np.float32)
    return np.float32(1.0) - np.float32(2.0) * nom / den


def kernel(output: np.ndarray, masks: np.ndarray) -> np.ndarray:
    loss, _ = run_split(output, masks)
    return loss


# revision 4
# speedup vs baseline: 1.0018x; 1.0018x over previous
"""Multi-class 3D DICE loss on 8 Trainium2 NeuronCores — reduced-precision
streaming.

The loss only needs three per-(subject, class) reductions over 1M-element
volumes: inter = sum(x*m), msum = sum(m), xsum = sum(x); the ~10-flop DICE
tail runs on the host. Random-rounding error on sums of 1M uniform values
averages out (~1e-6 relative on the loss, HW-verified), so the HBM stream —
the entire cost of this memory-bound kernel — runs below fp32:

  - x (probs) staged as bf16 (RNE, exact-format match with device decode)
  - m (masks) staged as bf16 or fp8e3 (E3M4; masks only feed sums and the
    product, never a divide) — M_FMT below.

HW-microbenchmarked facts this design is built on (see microbench.py):
  - Any DVE op with accum_out runs at 1x (the side accumulator path);
    ScalarE is always 1x; PE matmul is ~477-640ns per 512-col slice
    regardless of dtype. So each engine can carry ~one full reduction
    stream and the kernel must spread the three quantities across DVE
    (inter), ScalarE (msum), PE (xsum).
  - TRN2 has TWO hardware DGE queues (SP/sync + Activation/scalar).
    One queue saturates ~280 GB/s with engines busy; both together were
    measured ~480 GB/s. Chunks are greedily split across the queues by
    cumulative bytes. 16 KB/partition descriptors measured fastest.
  - Dummy stride-0 outputs for the accumulating ops save SBUF write
    bandwidth, which the DMA stream needs.

Sharding: 1024 units (one unit = 256 columns of the per-subject
[128, 32768] view) split by parity — even cores 127 units, odd cores 129 —
as two compiled variants on disjoint jax meshes; a sporadic late-stream DMA
stall only ever hits even cores, so evens get the smaller share. Each
core's shard is 1-3 contiguous segments, each inside one subject; segment
boundaries are placed so the big PSUM folds run mid-stream, off the tail.
The host regroups segments by subject and applies the DICE tail.
"""

import math
import os
import sys
from contextlib import ExitStack

import numpy as np

for _p in ("/opt/trn_rl_repo",):
    if _p not in sys.path and os.path.isdir(_p):
        sys.path.insert(0, _p)

import ml_dtypes  # noqa: E402

import concourse.tile as tile  # noqa: E402
from concourse import bacc, bass2jax, mybir  # noqa: E402

N_CORES = 8
B, C = 8, 4
SPATIAL = 64 * 128 * 128            # 1,048,576 per (subject, class)
P = 128                             # SBUF partitions = C * 32
SUBJ_COLS = (C * SPATIAL) // P      # 32768 columns per subject
UNIT = 256                          # shard granularity
SUBJ_UNITS = SUBJ_COLS // UNIT      # 128
MM = 512                            # matmul slice (max moving free dim)
EPS = 1e-7
F32 = mybir.dt.float32
BF16 = mybir.dt.bfloat16
U8 = mybir.dt.uint8

# Mask-tensor wire format: "bf16" (2 B/col) or "fp8e3" (1 B/col, E3M4).
M_FMT = "fp8e3"
X_BYTES = 2
M_BYTES = {"bf16": 2, "fp8e3": 1}[M_FMT]
BPC = X_BYTES + M_BYTES             # wire bytes per column (x + m)
M_DT = {"bf16": BF16, "fp8e3": mybir.dt.float8e3}[M_FMT]

# Variant name -> (chunk schedule in columns, chunks-per-segment). Chunk
# boundaries are aligned so no chunk straddles a segment boundary; 4096-col
# steady chunks give 16KB/partition descriptors (measured-best DMA rate)
# and a small run tail.
VARIANTS = {
    "vodd": dict(
        # spare unit FIRST; pyramid ramp (small chunks first so compute
        # starts early), 8192-col steady chunks (24KB descriptors), small
        # closing chunks; last segment tiny so its PSUM fold is cheap.
        chunks=[256, 1024, 2048, 4096, 8192, 8192, 4096, 2048, 1024, 512,
                512, 512, 512],
        seg_nchunks=(1, 11, 1),
    ),
    "veven": dict(
        chunks=[1024, 2048, 4096, 8192, 8192, 4096, 2048, 1024, 512, 512,
                512, 256],
        seg_nchunks=(11, 1),
    ),
}

# core -> (variant, [(subject, unit_start, n_units), ...]) in SEGMENT ORDER.
# The sporadic late-stream stall only ever hits EVEN cores; evens get 127
# units to odds' 129 so evens finish clear of it.
ASSIGN = {
    0: ("veven", [(0, 0, 126), (0, 126, 1)]),
    2: ("veven", [(2, 0, 126), (2, 126, 1)]),
    4: ("veven", [(4, 0, 126), (4, 126, 1)]),
    6: ("veven", [(6, 0, 126), (6, 126, 1)]),
    1: ("vodd", [(0, 127, 1), (1, 0, 126), (1, 126, 2)]),
    3: ("vodd", [(2, 127, 1), (3, 0, 126), (3, 126, 2)]),
    5: ("vodd", [(4, 127, 1), (5, 0, 126), (5, 126, 2)]),
    7: ("vodd", [(6, 127, 1), (7, 0, 126), (7, 126, 2)]),
}
GROUPS = {  # variant -> device ids; evens dispatched first (odds get the
    # later slot — dispatch-order overhead lands on the slack-rich group)
    "veven": [0, 2, 4, 6],
    "vodd": [1, 3, 5, 7],
}


def _queue_split(chunks):
    """Greedy split of chunk indices across the two HWDGE queues by bytes."""
    loads = [0, 0]
    which = []
    for fd in chunks:
        q = 0 if loads[0] <= loads[1] else 1
        which.append(q)
        loads[q] += fd
    return which


def _check_assign():
    cover = np.zeros((B, SUBJ_UNITS), dtype=int)
    for core, (vname, segs) in ASSIGN.items():
        v = VARIANTS[vname]
        starts = [sum(v["seg_nchunks"][:i]) for i in range(len(v["seg_nchunks"]))]
        seg_cols = [
            sum(v["chunks"][a : a + ns])
            for a, ns in zip(starts, v["seg_nchunks"])
        ]
        assert len(segs) == len(seg_cols)
        for (sub, us, n), cols in zip(segs, seg_cols):
            assert n * UNIT == cols, (core, vname, n * UNIT, cols)
            cover[sub, us : us + n] += 1
    assert (cover == 1).all()


_check_assign()


def _dice_body(ctx, tc, out_ap, x_ap, ind_ap, chunks, seg_nchunks):
    nc = tc.nc
    add = mybir.AluOpType.add
    mult = mybir.AluOpType.mult
    Copy = mybir.ActivationFunctionType.Copy
    NCH = len(chunks)
    NSEG = len(seg_nchunks)
    PADN = 8 * math.ceil(NCH / 8)  # keep each engine's accum cols in own 32B words
    seg_start = [sum(seg_nchunks[:i]) for i in range(NSEG)]  # first chunk of seg
    seg_of = []
    for s, ns in enumerate(seg_nchunks):
        seg_of += [s] * ns
    qs = _queue_split(chunks)

    def _n_slices(fd):
        return (fd + MM - 1) // MM

    seg_slices = [
        sum(_n_slices(c) for c in chunks[seg_start[s] : seg_start[s] + ns])
        for s, ns in enumerate(seg_nchunks)
    ]

    consts = ctx.enter_context(tc.tile_pool(name="consts", bufs=1))
    # Per-queue chunk pools: 3 buffers of run-ahead each.
    qpool = [
        ctx.enter_context(tc.tile_pool(name="xq0", bufs=3)),
        ctx.enter_context(tc.tile_pool(name="xq1", bufs=3)),
    ]
    small = ctx.enter_context(tc.tile_pool(name="small", bufs=1))
    psum = ctx.enter_context(tc.tile_pool(name="psum", bufs=1, space="PSUM"))

    # Block indicator: ind[q, c] = 1.0 iff q // 32 == c. f32 copy (memset)
    # for the final fp32 collapse; bf16 copy (DMA) as lhsT for the
    # per-chunk x-sum matmuls. Both exact.
    ind = consts.tile([P, C], F32)
    nc.vector.memset(ind[:], 0.0)
    for c in range(C):
        nc.vector.memset(ind[c * 32 : (c + 1) * 32, c : c + 1], 1.0)
    # Loaded after chunk 0's input DMA is issued — it is only needed by the
    # first matmul, which waits on chunk 0's data anyway.
    ind_b = consts.tile([P, C], BF16, tag="ind_b")

    # Per-chunk partial sums (column j <- chunk j); no cross-chunk deps.
    # Cols [0,PADN) = inter on DVE, [PADN,2*PADN) = msum on ScalarE — each
    # engine owns full 32 B accumulator words (mixing engines within one
    # word produced lost-update corruption on HW). Zero the pad columns so
    # the collapse matmul never reads uninitialized SBUF.
    acc = small.tile([P, 2 * PADN], F32)
    nc.vector.memset(acc[:], 0.0)
    # Accumulating ops run at 1x regardless of output AP (HW-measured), so
    # use stride-0 dummies and save the SBUF write bandwidth for DMA.
    dve_dummy = small.tile([P, 1], F32)
    act_dummy = small.tile([P, 1], F32)
    sums = small.tile([C, 3 * NSEG], F32, tag="sums")
    # PE accumulates per-class x column sums across each segment's slices.
    ps_x = []
    for s in range(NSEG):
        seg_cols = sum(chunks[seg_start[s] : seg_start[s] + seg_nchunks[s]])
        ps_x_s = psum.tile([C, min(MM, seg_cols)], F32, tag=f"ps_x{s}")
        ps_x.append(ps_x_s)

    off = 0
    sl_in_seg = 0
    for j, fd in enumerate(chunks):
        seg = seg_of[j]
        if j > 0 and seg_of[j - 1] != seg:
            sl_in_seg = 0
        # One fused DMA per chunk delivers [x_bytes | m_bytes]; chunks are
        # split across the two hardware DGE queues (sync + scalar).
        xmt = qpool[qs[j]].tile([P, BPC * fd], U8, tag="xmt")
        if qs[j] == 0:
            nc.sync.dma_start(out=xmt[:], in_=x_ap[:, BPC * off : BPC * (off + fd)])
        else:
            nc.scalar.dma_start(out=xmt[:], in_=x_ap[:, BPC * off : BPC * (off + fd)])
        xt = xmt[:, : X_BYTES * fd].bitcast(BF16)           # [P, fd] bf16
        mt = xmt[:, X_BYTES * fd :].bitcast(M_DT)           # [P, fd] m-format
        off += fd
        if j == 0:
            nc.sync.dma_start(out=ind_b[:], in_=ind_ap[:])

        # ScalarE: msum partial via accum_out (dummy out — ScalarE is 1x
        # regardless, and skipping the real write saves SBUF bandwidth).
        nc.scalar.activation(
            out=act_dummy.broadcast_to((P, fd)),
            in_=mt,
            func=Copy,
            accum_out=acc[:, PADN + j : PADN + j + 1],
        )

        # DVE: inter partial: accum = X-reduce((x*1)*m). Accumulating DVE
        # ops run at 1x, which handles mixed dtypes (fp8 m upconverts in
        # the fp32-internal datapath) — no ScalarE upcast needed.
        nc.vector.scalar_tensor_tensor(
            out=dve_dummy.broadcast_to((P, fd)),
            in0=xt,
            scalar=1.0,
            in1=mt,
            op0=mult,
            op1=mult,
            accum_out=acc[:, j : j + 1],
        )
        # x-sums on PE: ps_x[seg][c, i] += sum_q ind[q, c] * x[q, s*MM+i],
        # accumulated in PSUM across the segment's slices.
        for s in range(_n_slices(fd)):
            w = min(MM, fd - s * MM)
            nc.tensor.matmul(
                out=ps_x[seg][:, :w],
                lhsT=ind_b[:],
                rhs=xt[:, s * MM : s * MM + w],
                start=(sl_in_seg == 0),
                stop=(sl_in_seg == seg_slices[seg] - 1),
            )
            sl_in_seg += 1
        # Segment finished: fold its PSUM x-sums now, while the stream
        # continues — keeps the [C,512] reduce off the run tail.
        if j == seg_start[seg] + seg_nchunks[seg] - 1:
            nc.vector.tensor_reduce(
                sums[:, 3 * seg + 2 : 3 * seg + 3],
                ps_x[seg][:],
                axis=mybir.AxisListType.X,
                op=add,
            )

    # Partition blocks -> per-(class, quantity, chunk) sums in one matmul,
    # then per-segment PSUM-side reduces -> [C, 3*NSEG] segment sums
    # (inter, msum, xsum per segment). The remaining ~10-flop scalar tail
    # runs on the host during unshard.
    ps2 = psum.tile([C, 2 * PADN], F32)
    nc.tensor.matmul(out=ps2[:], lhsT=ind[:], rhs=acc[:], start=True, stop=True)
    for s, ns in enumerate(seg_nchunks):
        a = seg_start[s]
        nc.vector.tensor_reduce(
            sums[:, 3 * s : 3 * s + 1],
            ps2[:, a : a + ns],
            axis=mybir.AxisListType.X,
            op=add,
        )
        nc.vector.tensor_reduce(
            sums[:, 3 * s + 1 : 3 * s + 2],
            ps2[:, PADN + a : PADN + a + ns],
            axis=mybir.AxisListType.X,
            op=add,
        )
    nc.sync.dma_start(out=out_ap, in_=sums[:])


_CACHE: dict[str, object] = {}


def _build(vname: str):
    key = f"nc_{vname}"
    if key in _CACHE:
        return _CACHE[key]
    v = VARIANTS[vname]
    cols = sum(v["chunks"])
    nseg = len(v["seg_nchunks"])
    nc = bacc.Bacc("TRN2", target_bir_lowering=False, debug=False)
    xm = nc.dram_tensor("xm", [P, BPC * cols], U8, kind="ExternalInput").ap()
    ind = nc.dram_tensor("ind", [P, C], BF16, kind="ExternalInput").ap()
    out = nc.dram_tensor("seg_sums", [C, 3 * nseg], F32, kind="ExternalOutput").ap()
    with tile.TileContext(nc) as tc:
        with ExitStack() as ctx:
            _dice_body(ctx, tc, out, xm, ind, v["chunks"], v["seg_nchunks"])
    nc.compile()
    _CACHE[key] = nc
    return nc


def _runner(vname: str):
    """Jitted shard_map runner for a variant on its assigned devices."""
    key = f"run_{vname}"
    if key in _CACHE:
        return _CACHE[key]
    import jax
    from jax.experimental.shard_map import shard_map
    from jax.sharding import Mesh, PartitionSpec

    bass2jax.install_neuronx_cc_hook()
    nc = _build(vname)
    device_ids = GROUPS[vname]

    partition_name = (
        nc.partition_id_tensor.name if nc.partition_id_tensor else None
    )
    in_names, out_names, out_avals, zero_outs = [], [], [], []
    for alloc in nc.m.functions[0].allocations:
        if not isinstance(alloc, mybir.MemoryLocationSet):
            continue
        name = alloc.memorylocations[0].name
        if alloc.kind == "ExternalInput":
            if name != partition_name:
                in_names.append(name)
        elif alloc.kind == "ExternalOutput":
            out_names.append(name)
            shape = tuple(alloc.tensor_shape)
            dtype = mybir.dt.np(alloc.dtype)
            out_avals.append(jax.core.ShapedArray(shape, dtype))
            zero_outs.append(np.zeros(shape, dtype))
    n_params = len(in_names)
    n_outs = len(out_avals)
    all_in_names = in_names + out_names
    if partition_name is not None:
        all_in_names.append(partition_name)
    donate = tuple(range(n_params, n_params + n_outs))

    def _body(*args):
        operands = list(args)
        if partition_name is not None:
            operands.append(bass2jax.partition_id_tensor())
        outs = bass2jax._bass_exec_p.bind(
            *operands,
            out_avals=tuple(out_avals),
            in_names=tuple(all_in_names),
            out_names=tuple(out_names),
            lowering_input_output_aliases=(),
            sim_require_finite=True,
            sim_require_nnan=True,
            nc=nc,
        )
        return tuple(outs)

    devices = [jax.devices()[i] for i in device_ids]
    n = len(devices)
    mesh = Mesh(np.asarray(devices), ("core",))
    in_specs = (PartitionSpec("core"),) * (n_params + n_outs)
    out_specs = (PartitionSpec("core"),) * n_outs
    sharded = jax.jit(
        shard_map(_body, mesh=mesh, in_specs=in_specs, out_specs=out_specs,
                  check_rep=False),
        donate_argnums=donate,
        keep_unused=True,
    )

    def run(in_maps):
        assert len(in_maps) == n
        per_core = [[np.asarray(m_[nm]) for nm in in_names] for m_ in in_maps]
        concat_in = [
            np.concatenate([per_core[c][i] for c in range(n)], axis=0)
            for i in range(n_params)
        ]
        concat_zeros = [
            np.zeros((n * z.shape[0], *z.shape[1:]), z.dtype) for z in zero_outs
        ]
        out_arrs = sharded(*concat_in, *concat_zeros)

        def gather():
            return [
                {
                    name: np.asarray(out_arrs[i]).reshape(n, *out_avals[i].shape)[c]
                    for i, name in enumerate(out_names)
                }
                for c in range(n)
            ]

        return gather

    _CACHE[key] = run
    return run


_IND_NP = np.repeat(np.eye(C, dtype=np.float32), 32, axis=0).astype(
    ml_dtypes.bfloat16
)  # [128, 4]


def _stage(output: np.ndarray, masks: np.ndarray):
    """Whole-tensor dtype staging, done once per kernel() call."""
    x16 = (
        np.ascontiguousarray(output, dtype=np.float32)
        .astype(ml_dtypes.bfloat16)
        .reshape(B, P, SUBJ_COLS)
    )
    m_np = {"bf16": ml_dtypes.bfloat16, "fp8e3": ml_dtypes.float8_e3m4}[M_FMT]
    m8 = (
        np.ascontiguousarray(masks, dtype=np.float32)
        .astype(m_np)
        .reshape(B, P, SUBJ_COLS)
    )
    return x16, m8


def _core_inputs(x16: np.ndarray, m8: np.ndarray, core: int):
    vname, segs = ASSIGN[core]
    xs, ms = [], []
    for sub, us, n in segs:
        lo, hi = us * UNIT, (us + n) * UNIT
        xs.append(x16[sub, :, lo:hi])
        ms.append(m8[sub, :, lo:hi])
    x = np.ascontiguousarray(np.concatenate(xs, axis=1)).view(np.uint8)
    m = np.ascontiguousarray(np.concatenate(ms, axis=1)).view(np.uint8)
    # Pack per DMA chunk: [x_chunk_bytes | m_chunk_bytes] so one transfer
    # feeds both operands.
    chunks = VARIANTS[vname]["chunks"]
    cols = x.shape[1] // X_BYTES
    xm = np.empty((P, BPC * cols), dtype=np.uint8)
    off = 0
    for fd in chunks:
        dst = BPC * off
        xm[:, dst : dst + X_BYTES * fd] = x[:, X_BYTES * off : X_BYTES * (off + fd)]
        xm[:, dst + X_BYTES * fd : dst + BPC * fd] = m[
            :, M_BYTES * off : M_BYTES * (off + fd)
        ]
        off += fd
    return {"xm": xm, "ind": _IND_NP}


def run_split(output: np.ndarray, masks: np.ndarray):
    """Dispatch both variants concurrently; returns (loss[1], groups)
    where groups = [(vname, nc, device_ids)] for the profiler."""
    x16, m8 = _stage(output, masks)

    def _dispatch_all():
        gathers = []
        for vname, ids in GROUPS.items():
            run = _runner(vname)
            gathers.append(
                (vname, ids, run([_core_inputs(x16, m8, c) for c in ids]))
            )
        # force completion inside the retry scope
        return [(v, ids, g()) for v, ids, g in gathers]

    try:
        finished = _dispatch_all()
    except Exception:  # e.g. a wedged NeuronCore from a prior run — retry once
        import time as _time

        _time.sleep(10)
        finished = _dispatch_all()

    # [B, C, 3] per-subject class sums assembled from segment partials.
    subj = np.zeros((B, C, 3), dtype=np.float32)
    for vname, ids, results in finished:
        for slot, core in enumerate(ids):
            _, segs = ASSIGN[core]
            seg_sums = results[slot]["seg_sums"].astype(np.float32)  # [C, 3*NSEG]
            for s, (sub, _, _) in enumerate(segs):
                subj[sub] += seg_sums[:, 3 * s : 3 * s + 3]

    per_subj = np.array([_finish(subj[b]) for b in range(B)], dtype=np.float32)
    loss = (per_subj.sum(dtype=np.float32) / np.float32(B)).reshape(1)
    groups = [(vname, _CACHE[f"nc_{vname}"], ids) for vname, ids in GROUPS.items()]
    return loss.astype(np.float32), groups


def _finish(cs: np.ndarray) -> np.float32:
    """Per-subject scalar tail (fp32, mirrors the reference ordering).

    cs: [C, 3] — columns (inter, mask_sum, x_sum) per class.
    """
    cs = cs.astype(np.float32)
    inter, msum, xsum = cs[:, 0], cs[:, 1], cs[:, 2]
    w = np.float32(1.0) / (msum * msum + np.float32(EPS))
    total = xsum + msum
    nom = (w * inter).sum(dtype=np.float32)
    den = (w * total + np.float32(EPS)).sum(dtype=np.float32)
    return np.float32(1.0) - np.float32(2.0) * nom / den


def kernel(output: np.ndarray, masks: np.ndarray) -> np.ndarray:
    loss, _ = run_split(output, masks)
    return loss
